# revision 19
# baseline (speedup 1.0000x reference)
"""3D Gaussian Splat renderer on 8 TRN2 NeuronCores.

Strategy (per spec sharding hint): host does the tiny O(N) per-gaussian work
(projection, 2x2 covariance inverse, depth sort, quadratic-form coefficients);
the sorted gaussians are replicated to all 8 cores and each core rasterizes a
32-row band of the 256x256 image.

Device algorithm per core (16 pixel-tiles of 512 px, 8 depth-blocks of 128):
  power  = G_b^T @ P_t               one fp32 matmul, K=6 monomial basis
  alpha  = Exp(power)                ScalarE (power has log-opacity folded in)
  L      = Ln(1 - alpha)  -> bf16    ScalarE (same ACT table set as Exp)
  carry  = colsum PSUM accumulator   ones-vector bf16 matmul per block
  scan   = strict-upper-tri bf16 matmul, accumulated onto the power PSUM,
           plus a rank-2 (hi/lo split of carry) bf16 matmul for the
           cross-block transmittance carry
  w      = Exp(power + scan + carry) -> bf16
  img   += col_b^T @ w               K=128 -> M=3 bf16 matmul, PSUM-accumulated
"""

import numpy as np
import ml_dtypes

N, H, W = 1024, 256, 256
NEAR, MIN_COV = 1e-4, 1e-4
NCORES = 8
ROWS_PER_CORE = H // NCORES          # 32
PIX_PER_CORE = ROWS_PER_CORE * W     # 8192
F = 512                              # pixels per tile (one PSUM bank, fp32)
NT = PIX_PER_CORE // F               # 16 tiles
NB = N // 128                        # 8 gaussian blocks

BF16 = ml_dtypes.bfloat16


def _host_precompute(means, log_scales, colors, opacities, intrinsics,
                     camera_to_world):
    """Projection, sort, and per-gaussian polynomial coefficients (float64)."""
    means = np.asarray(means, np.float64)
    log_scales = np.asarray(log_scales, np.float64)
    colors = np.asarray(colors, np.float64)
    opacities = np.asarray(opacities, np.float64)
    K = np.asarray(intrinsics, np.float64)
    c2w = np.asarray(camera_to_world, np.float64)

    scales = np.exp(log_scales)
    cov3 = np.zeros((N, 3, 3))
    cov3[:, np.arange(3), np.arange(3)] = scales * scales
    cov3 += np.eye(3) * 1e-6
    R = c2w[:3, :3]
    t = c2w[:3, 3]
    Rw2c = R.T
    tw2c = -Rw2c @ t
    mc = means @ Rw2c.T + tw2c
    cov_cam = np.einsum('ij,njk,lk->nil', Rw2c, cov3, Rw2c)
    x, y, z = mc[:, 0], mc[:, 1], mc[:, 2]
    vis = z > NEAR
    sz = np.where(vis, z, 1.0)
    fx, fy, cx, cy = K[0, 0], K[1, 1], K[0, 2], K[1, 2]
    px = fx * x / sz + cx
    py = fy * y / sz + cy
    zero = np.zeros_like(sz)
    J = np.stack([np.stack([fx / sz, zero, -fx * x / (sz * sz)], -1),
                  np.stack([zero, fy / sz, -fy * y / (sz * sz)], -1)], 1)
    cov2 = np.einsum('nij,njk,nlk->nil', J, cov_cam, J) + np.eye(2) * MIN_COV
    mask = vis & (px >= 0) & (px < W) & (py >= 0) & (py < H)
    order = np.argsort(np.where(mask, z, np.inf), kind='stable')
    px, py, cov2, mask = px[order], py[order], cov2[order], mask[order]
    col = np.clip(colors, 0, 1)[order]
    opac = (1.0 / (1.0 + np.exp(-opacities)))[order]

    a = cov2[:, 0, 0]
    b = cov2[:, 0, 1]
    c = cov2[:, 1, 1]
    det = a * c - b * b
    ia, ib, ic = c / det, -b / det, a / det
    # power(x,y) = A x^2 + B y^2 + C xy + D x + E y + F0, log(opac) folded in
    A = -0.5 * ia
    B = -0.5 * ic
    C = -ib
    D = ia * px + ib * py
    E = ic * py + ib * px
    F0 = -0.5 * (ia * px * px + ic * py * py + 2 * ib * px * py) + np.log(opac)
    F0 = np.where(mask, F0, -1e4)  # culled gaussians: alpha == 0 everywhere
    G = np.stack([A, B, C, D, E, F0], 0).astype(np.float32)        # (6, N)
    # color matmul lhsT, grouped per block: colb[p, b*3+c] = col[b*128+p, c]
    colb = col.astype(np.float32).reshape(NB, 128, 3).transpose(1, 0, 2) \
              .reshape(128, NB * 3).astype(BF16)
    return G, colb


def _pixel_basis(core):
    """Monomial basis (6, 8192) for this core's 32 image rows."""
    rows = np.arange(core * ROWS_PER_CORE, (core + 1) * ROWS_PER_CORE,
                     dtype=np.float32)
    yy = np.repeat(rows, W)
    xx = np.tile(np.arange(W, dtype=np.float32), ROWS_PER_CORE)
    return np.stack([xx * xx, yy * yy, xx * yy, xx, yy,
                     np.ones_like(xx)], 0)


_STATE = {}


def _build(f32r=True, ppow_bufs=3, pscn_bufs=2, pimg_bufs=1, work_bufs=3,
           rowp_bufs=4, reps=1):
    """Build + compile the SPMD Bass program (once per process)."""
    key = (f32r, ppow_bufs, pscn_bufs, pimg_bufs, work_bufs, rowp_bufs, reps)
    if _STATE.get('key') == key:
        return _STATE['nc']
    from contextlib import ExitStack
    import concourse.bass as bass  # noqa: F401
    import concourse.bacc as bacc
    import concourse.mybir as mybir
    import concourse.tile as tile

    f32 = mybir.dt.float32
    bf16 = mybir.dt.bfloat16
    AF = mybir.ActivationFunctionType

    nc = bacc.Bacc("TRN2", target_bir_lowering=False, debug=False,
                   num_devices=NCORES)
    mmdt = mybir.dt.float32r if f32r else f32
    g6t_d = nc.dram_tensor("g6t", [6, N], mmdt, kind="ExternalInput").ap()
    pb_d = nc.dram_tensor("pb", [6, PIX_PER_CORE], mmdt,
                          kind="ExternalInput").ap()
    colb_d = nc.dram_tensor("colb", [128, NB * 3], bf16,
                            kind="ExternalInput").ap()
    ut_d = nc.dram_tensor("ut", [128, 128], bf16, kind="ExternalInput").ap()
    on1_d = nc.dram_tensor("on1", [128, 1], bf16, kind="ExternalInput").ap()
    on2_d = nc.dram_tensor("on2", [1, 128], bf16, kind="ExternalInput").ap()
    out_d = nc.dram_tensor("out", [3, PIX_PER_CORE], f32,
                           kind="ExternalOutput").ap()

    with tile.TileContext(nc) as tc, ExitStack() as ctx:
        const = ctx.enter_context(tc.tile_pool(name="const", bufs=1))
        work = ctx.enter_context(tc.tile_pool(name="work", bufs=work_bufs))
        rowp = ctx.enter_context(tc.tile_pool(name="rowp", bufs=rowp_bufs))
        ppow = ctx.enter_context(tc.tile_pool(name="ppow", bufs=ppow_bufs,
                                              space="PSUM"))
        pscn = ctx.enter_context(tc.tile_pool(name="pscn", bufs=pscn_bufs,
                                              space="PSUM"))
        ptot = ctx.enter_context(tc.tile_pool(name="ptot", bufs=2,
                                              space="PSUM"))
        pimg = ctx.enter_context(tc.tile_pool(name="pimg", bufs=pimg_bufs,
                                              space="PSUM"))

        g6t = const.tile([6, N], mmdt)
        nc.sync.dma_start(g6t[:], g6t_d)
        pb = const.tile([6, PIX_PER_CORE], mmdt)
        nc.sync.dma_start(pb[:], pb_d)
        colb = const.tile([128, NB * 3], bf16)
        nc.sync.dma_start(colb[:], colb_d)
        ut = const.tile([128, 128], bf16)
        nc.sync.dma_start(ut[:], ut_d)
        on1 = const.tile([128, 1], bf16)
        nc.sync.dma_start(on1[:], on1_d)
        on2 = const.tile([1, 128], bf16)
        nc.sync.dma_start(on2[:], on2_d)
        out_sb = const.tile([3, PIX_PER_CORE], f32)

        # Emission is phase-grouped per tile so every engine's in-order
        # stream gets independent work back-to-back (block b's alpha does
        # not wait on block b-1's scan, etc.).
        for t in [tt % NT for tt in range(NT * reps)]:
            Pt = pb[:, t * F:(t + 1) * F]
            img = pimg.tile([3, F], f32, tag="img")
            # phase 1: power matmuls + Exp, pipelined through 2 PSUM bufs
            alpha = work.tile([128, NB * F], f32, tag="alpha")
            for b in range(NB):
                pw = ppow.tile([128, F], f32, tag="pow")
                nc.tensor.matmul(pw[:], g6t[:, b * 128:(b + 1) * 128], Pt,
                                 start=True, stop=True)
                nc.scalar.activation(alpha[:, b * F:(b + 1) * F], pw[:],
                                     AF.Exp)
            # phase 2: L = Ln(1 - alpha), bf16 (per-block ops pipeline best)
            La = work.tile([128, NB * F], bf16, tag="L")
            for b in range(NB):
                s = slice(b * F, (b + 1) * F)
                nc.scalar.activation(La[:, s], alpha[:, s], AF.Ln,
                                     bias=1.0, scale=-1.0)
            # phase 3: block colsums + serial carry chain (DVE rows)
            cars = []
            car = None
            for b in range(NB - 1):
                tb = ptot.tile([1, F], f32, tag="tot")
                nc.tensor.matmul(tb[:], on1[:], La[:, b * F:(b + 1) * F],
                                 start=True, stop=True)
                nxt = rowp.tile([1, F], f32, tag="car")
                if b == 0:
                    nc.vector.tensor_copy(nxt[:], tb[:])
                else:
                    nc.vector.tensor_add(nxt[:], car[:], tb[:])
                car = nxt
                chl = rowp.tile([1, F], bf16, tag="chl")
                nc.vector.tensor_copy(chl[:], car[:])
                cars.append(chl)
            # phase 4: carry + scan + (power again) into one PSUM group,
            # then w = Exp(power + scan + carry) directly -- no DVE multiply
            wa = work.tile([128, NB * F], bf16, tag="w")
            for b in range(NB):
                ps = pscn.tile([128, F], f32, tag="scan")
                first = True
                if b > 0:
                    nc.tensor.matmul(ps[:], on2[:], cars[b - 1][:],
                                     start=True, stop=False)
                    first = False
                nc.tensor.matmul(ps[:], ut[:], La[:, b * F:(b + 1) * F],
                                 start=first, stop=False)
                nc.tensor.matmul(ps[:], g6t[:, b * 128:(b + 1) * 128], Pt,
                                 start=False, stop=True)
                nc.scalar.activation(wa[:, b * F:(b + 1) * F], ps[:], AF.Exp)
            # phase 5: color matmuls accumulate
            for b in range(NB):
                nc.tensor.matmul(img[:], colb[:, b * 3:(b + 1) * 3],
                                 wa[:, b * F:(b + 1) * F], start=(b == 0),
                                 stop=(b == NB - 1))
            nc.vector.tensor_copy(out_sb[:, t * F:(t + 1) * F], img[:])
        nc.sync.dma_start(out_d, out_sb[:])

    nc.compile()
    _STATE['nc'] = nc
    _STATE['key'] = key
    return nc


def _in_maps(inputs):
    G, colb = _host_precompute(**inputs)
    ut = np.triu(np.ones((128, 128), np.float32), 1).astype(BF16)
    on1 = np.ones((128, 1), BF16)
    on2 = np.ones((1, 128), BF16)
    return [{
        "g6t": G,
        "pb": _pixel_basis(c),
        "colb": colb,
        "ut": ut,
        "on1": on1,
        "on2": on2,
    } for c in range(NCORES)]


def _gather(results):
    full = np.zeros((H, W, 3), np.float32)
    for c in range(NCORES):
        o = np.asarray(results[c]["out"])  # (3, 8192)
        full[c * ROWS_PER_CORE:(c + 1) * ROWS_PER_CORE] = \
            o.reshape(3, ROWS_PER_CORE, W).transpose(1, 2, 0)
    return full


def _run(inputs, trace=False):
    from concourse.bass_utils import run_bass_kernel_spmd
    nc = _build()
    res = run_bass_kernel_spmd(nc, _in_maps(inputs), list(range(NCORES)),
                               trace=trace)
    return _gather(res.results), res


def kernel(**inputs):
    out, _ = _run(inputs, trace=False)
    return out
